# revision 3
# baseline (speedup 1.0000x reference)
"""M2MRF module on 8 TRN2 NeuronCores -- single collapsed GEMM.

fold(W2 @ (W1 @ unfold(x))) has no nonlinearity between the two 1x1-conv
GEMMs, so the chain collapses algebraically to one GEMM with
Wc = W2 @ W1 ([256, 1024]), computed exactly on the host (0.27 GFLOP).
Per core (4 batches x 2 L-halves): y2 = Wc @ cols_half
([256,1024] @ [1024,8192] bf16, fp32 PSUM accumulate) -- 4.3 GFLOP at the
Tensor-engine roofline, balanced against 21 MB of DMA on the 360 GB/s bus.

Device schedule per core:
  - DRAM layouts are partition-major so every DMA is 128 fat descriptors.
  - weights stream on the SP HWDGE queue (half0 in two k-pieces so the
    first matmul only waits for 1/4 of the weights);
  - ALL x tiles stream on the Pool SWDGE queue: its 25ns sequencer
    dispatch gives back-to-back bus transfers (no HWDGE turnaround), and
    output DMAs naturally queue behind the input stream on the shared bus,
    so the PE is never starved (no p-state re-ramp);
  - PE: 16 n-tiles x (2 m-chunks x 8 k-chunk matmuls), PSUM accumulate,
    tile0 k-sliced for an early start; DVE casts PSUM->SBUF bf16;
  - outputs ride the Act HWDGE queue; the last two tiles are split into
    small pieces spread across the Act/SP queues to shorten the tail.
"""
import sys

sys.path.insert(0, "/opt/trn_rl_repo")

import numpy as np
import ml_dtypes

import concourse.bass as bass
import concourse.bacc as bacc
import concourse.mybir as mybir
import concourse.tile as tile
from concourse.bass_utils import run_bass_kernel_spmd

P = 128
NT = 512            # free-dim tile (one PSUM bank of fp32)
LSH = 8192          # L per core
NTILES = LSH // NT  # 16
KC = 8              # 1024 / 128 contraction chunks
COUT = 256
PAD = 4             # o_all pad columns
NTP = NT + PAD

_BF16 = ml_dtypes.bfloat16


def _build_nc(t0_split=4, t1_split=2, t2_split=2, n_dummy=0, memset_pad=True,
              out_q="scalar", tail_qs=("scalar", "sync", "scalar", "sync"),
              last_cols=256, pool_copy=False):
    """v8: partition-major DRAM layouts (128 fat descriptors per DMA);
    weights on the SP HWDGE queue; ALL x input tiles on the Pool SWDGE
    queue (25ns seq dispatch -> back-to-back bus transfers, no 286ns
    HWDGE turnarounds); outs on Act HWDGE; tail pieces spread
    across Act/SP; optional live dummy matmuls on the weights to start
    the PE p-state ramp before the first x data lands."""
    nc = bacc.Bacc("TRN2", target_bir_lowering=False)
    xin = nc.dram_tensor("xin", [NTILES, P, KC, NT], mybir.dt.bfloat16,
                         kind="ExternalInput")
    wct = nc.dram_tensor("wct", [2, P, KC, P], mybir.dt.bfloat16,
                         kind="ExternalInput")
    y2 = nc.dram_tensor("y2", [NTILES, P, 2, NTP], mybir.dt.bfloat16,
                        kind="ExternalOutput")

    with tile.TileContext(nc) as tc:
        with (
            tc.tile_pool(name="res", bufs=1) as res,
            tc.tile_pool(name="ps", bufs=7, space="PSUM") as ps,
            tc.tile_pool(name="psw", bufs=1, space="PSUM") as psw,
        ):
            wc_sb = res.tile([P, 2, KC, P], mybir.dt.bfloat16, tag="wc")
            x_sb = res.tile([P, NTILES, KC, NT], mybir.dt.bfloat16, tag="x")
            o_all = res.tile([P, NTILES, 2, NTP], mybir.dt.bfloat16, tag="o")
            if memset_pad:
                nc.vector.memset(o_all[:, :, :, NT:], 0.0)

            # ---- weights on SP HWDGE (half0 in two k-pieces so the first
            # matmul only waits for k0-3)
            nc.sync.dma_start(wc_sb[:, 0, :KC // 2], wct.ap()[0, :, :KC // 2])
            nc.sync.dma_start(wc_sb[:, 0, KC // 2:], wct.ap()[0, :, KC // 2:])
            nc.sync.dma_start(wc_sb[:, 1], wct.ap()[1])
            # ---- x stream entirely on Pool SWDGE, back-to-back
            for nt in range(NTILES):
                split = (t0_split if nt == 0 else t1_split if nt == 1
                         else t2_split if nt == 2 else 1)
                kstep = KC // split
                for h in range(split):
                    ks = slice(h * kstep, (h + 1) * kstep)
                    nc.gpsimd.dma_start(x_sb[:, nt, ks, :], xin.ap()[nt, :, ks])

            # ---- dummy ramp starters on the weights (live via pad copy)
            if n_dummy:
                pw = psw.tile([P, P], mybir.dt.float32, tag="warm")
                mv = wc_sb[:, 0].rearrange("p k m -> p (k m)")
                for i in range(n_dummy):
                    nc.tensor.matmul(pw[:], wc_sb[:, 0, 0, :], mv[:, :P],
                                     start=(i == 0), stop=(i == n_dummy - 1))
                nc.vector.tensor_copy(o_all[:, 0, 0, NT + 1:NT + 2], pw[:, :1])

            # ---- compute
            for nt in range(NTILES):
                last = nt == NTILES - 1
                pieces = ([(0, NT)] if not last else
                          [(0, NT - last_cols), (NT - last_cols, NT)])
                for (c0, c1) in pieces:
                    for m2 in range(2):
                        pt = ps.tile([P, NT], mybir.dt.float32, tag="ps")
                        for k in range(KC):
                            nc.tensor.matmul(
                                pt[:, :c1 - c0],
                                wc_sb[:, m2, k, :],
                                x_sb[:, nt, k, c0:c1],
                                start=(k == 0),
                                stop=(k == KC - 1),
                            )
                        if pool_copy and last and m2 == 1:
                            nc.scalar.copy(
                                o_all[:, nt, m2, c0:c1], pt[:, :c1 - c0])
                        else:
                            nc.vector.tensor_copy(
                                o_all[:, nt, m2, c0:c1], pt[:, :c1 - c0])

            # ---- output DMAs: partition-major dest, per-tile on Act
            oq = {"scalar": nc.scalar, "sync": nc.sync,
                  "pool": nc.gpsimd}[out_q]
            tq = [{"pool": nc.gpsimd, "scalar": nc.scalar,
                   "sync": nc.sync}[q] for q in tail_qs]
            ti = 0
            for nt in range(NTILES):
                last = nt == NTILES - 1
                if nt >= NTILES - 2:
                    pieces = ([(0, NTP)] if not last else
                              [(0, NT - last_cols), (NT - last_cols, NTP)])
                    for (c0, c1) in pieces:
                        for m2 in range(2):
                            tq[ti % len(tq)].dma_start(
                                y2.ap()[nt, :, m2, c0:c1],
                                o_all[:, nt, m2, c0:c1])
                            ti += 1
                else:
                    oq.dma_start(y2.ap()[nt], o_all[:, nt])

    nc.finalize()
    return nc

_NC_CACHE = None


def kernel(x, W1, b1, W2, b2):
    global _NC_CACHE
    x = np.asarray(x)
    W1, b1 = np.asarray(W1), np.asarray(b1)
    W2, b2 = np.asarray(W2), np.asarray(b2)
    n, c, h, w = x.shape  # 4, 64, 512, 512

    # ---- host: collapse the two linear maps exactly
    Wc = (W2.astype(np.float64) @ W1.astype(np.float64)).astype(np.float32)

    # ---- host unfold: cols[b, c*16+kh*4+kw, ph*128+pw] = x[b,c,ph*4+kh,pw*4+kw]
    xb = x.astype(_BF16)
    cols = xb.reshape(n, c, 128, 4, 128, 4).transpose(0, 1, 3, 5, 2, 4)
    cols = np.ascontiguousarray(cols).reshape(n, 1024, 16384)

    if _NC_CACHE is None:
        _NC_CACHE = _build_nc()
    nc = _NC_CACHE

    # wct[m2, p, k, m] = Wc[m2*128+m, k*128+p]
    wct6 = np.ascontiguousarray(
        Wc.astype(_BF16).reshape(2, P, KC, P).transpose(0, 3, 2, 1))

    in_maps = []
    for core in range(8):
        b, half = core // 2, core % 2
        # [1024, 8192] -> [16 nt, 128 p, 8 k, 512] (partition-major)
        xc = cols[b, :, half * LSH:(half + 1) * LSH]
        xc = np.ascontiguousarray(
            xc.reshape(KC, P, NTILES, NT).transpose(2, 1, 0, 3))
        in_maps.append({"xin": xc, "wct": wct6})

    res = run_bass_kernel_spmd(nc, in_maps, core_ids=list(range(8)))

    # ---- gather + fold on host
    y2 = np.empty((n, COUT, 16384), dtype=np.float32)
    for core in range(8):
        b, half = core // 2, core % 2
        yc = np.asarray(res.results[core]["y2"])[:, :, :, :NT]  # strip pad
        # [16 nt, 128 p, 2 m2, 512] -> [m2, p, nt, n]
        yc = yc.astype(np.float32).transpose(2, 1, 0, 3)
        y2[b, :, half * LSH:(half + 1) * LSH] = yc.reshape(COUT, LSH)

    # bias epilogue (b1/b2 are zeros in this problem; exact otherwise)
    v = W2.astype(np.float64) @ b1.astype(np.float64) + b2.astype(np.float64)
    if np.any(v):
        y2 += v.astype(np.float32)[None, :, None]

    out = y2.reshape(n, c, 2, 2, 128, 128).transpose(0, 1, 4, 2, 5, 3)
    return np.ascontiguousarray(out).reshape(n, c, 256, 256)



# revision 5
# speedup vs baseline: 1.0048x; 1.0048x over previous
"""M2MRF module on 8 TRN2 NeuronCores -- single collapsed GEMM.

fold(W2 @ (W1 @ unfold(x))) has no nonlinearity between the two 1x1-conv
GEMMs, so the chain collapses algebraically to one GEMM with
Wc = W2 @ W1 ([256, 1024]), computed exactly on the host (0.27 GFLOP).
Per core (4 batches x 2 L-halves): y2 = Wc @ cols_half
([256,1024] @ [1024,8192] bf16, fp32 PSUM accumulate) -- 4.3 GFLOP at the
Tensor-engine roofline, balanced against 21 MB of DMA on the 360 GB/s bus.

Device schedule per core:
  - DRAM layouts are partition-major so every DMA is 128 fat descriptors.
  - weights stream on the SP HWDGE queue (half0 in two k-pieces so the
    first matmul only waits for 1/4 of the weights);
  - ALL x tiles stream on the Pool SWDGE queue: its 25ns sequencer
    dispatch gives back-to-back bus transfers (no HWDGE turnaround), and
    output DMAs naturally queue behind the input stream on the shared bus,
    so the PE is never starved (no p-state re-ramp);
  - PE: 16 n-tiles x (2 m-chunks x 8 k-chunk matmuls), PSUM accumulate,
    tile0 k-sliced for an early start; DVE casts PSUM->SBUF bf16;
  - outputs ride the Act HWDGE queue; the last two tiles are split into
    small pieces spread across the Act/SP queues to shorten the tail.
"""
import sys

sys.path.insert(0, "/opt/trn_rl_repo")

import numpy as np
import ml_dtypes

import concourse.bass as bass
import concourse.bacc as bacc
import concourse.mybir as mybir
import concourse.tile as tile
from concourse.bass_utils import run_bass_kernel_spmd

P = 128
NT = 512            # free-dim tile (one PSUM bank of fp32)
LSH = 8192          # L per core
NTILES = LSH // NT  # 16
KC = 8              # 1024 / 128 contraction chunks
COUT = 256
PAD = 4             # o_all pad columns
NTP = NT + PAD

_BF16 = ml_dtypes.bfloat16


def _build_nc(t0_split=4, split_upto=7, n_dummy=0,
              out_q="scalar", tail_qs=("scalar", "sync", "scalar", "sync"),
              last_cols=256, pool_copy=False):
    """v8: partition-major DRAM layouts (128 fat descriptors per DMA);
    weights on the SP HWDGE queue; ALL x input tiles on the Pool SWDGE
    queue (25ns seq dispatch -> back-to-back bus transfers, no 286ns
    HWDGE turnarounds); outs on Act HWDGE; tail pieces spread
    across Act/SP; optional live dummy matmuls on the weights to start
    the PE p-state ramp before the first x data lands."""
    nc = bacc.Bacc("TRN2", target_bir_lowering=False)
    xin = nc.dram_tensor("xin", [NTILES, P, KC, NT], mybir.dt.bfloat16,
                         kind="ExternalInput")
    wct = nc.dram_tensor("wct", [2, P, KC, P], mybir.dt.bfloat16,
                         kind="ExternalInput")
    y2 = nc.dram_tensor("y2", [NTILES, P, 2, NT], mybir.dt.bfloat16,
                        kind="ExternalOutput")

    with tile.TileContext(nc) as tc:
        with (
            tc.tile_pool(name="res", bufs=1) as res,
            tc.tile_pool(name="ps", bufs=7, space="PSUM") as ps,
            tc.tile_pool(name="psw", bufs=1, space="PSUM") as psw,
        ):
            wc_sb = res.tile([P, 2, KC, P], mybir.dt.bfloat16, tag="wc")
            x_sb = res.tile([P, NTILES, KC, NT], mybir.dt.bfloat16, tag="x")
            o_all = res.tile([P, NTILES, 2, NT], mybir.dt.bfloat16, tag="o")

            # ---- weights on SP HWDGE (half0 in two k-pieces so the first
            # matmul only waits for k0-3)
            nc.sync.dma_start(wc_sb[:, 0, :KC // 2], wct.ap()[0, :, :KC // 2])
            nc.sync.dma_start(wc_sb[:, 0, KC // 2:], wct.ap()[0, :, KC // 2:])
            nc.sync.dma_start(wc_sb[:, 1], wct.ap()[1])
            # ---- x stream entirely on Pool SWDGE, back-to-back
            for nt in range(NTILES):
                split = (t0_split if nt == 0
                         else 2 if nt <= split_upto else 1)
                kstep = KC // split
                for h in range(split):
                    ks = slice(h * kstep, (h + 1) * kstep)
                    nc.gpsimd.dma_start(x_sb[:, nt, ks, :], xin.ap()[nt, :, ks])


            # ---- compute
            for nt in range(NTILES):
                last = nt == NTILES - 1
                pieces = ([(0, NT)] if not last else
                          [(0, NT - last_cols), (NT - last_cols, NT)])
                for (c0, c1) in pieces:
                    for m2 in range(2):
                        pt = ps.tile([P, NT], mybir.dt.float32, tag="ps")
                        for k in range(KC):
                            nc.tensor.matmul(
                                pt[:, :c1 - c0],
                                wc_sb[:, m2, k, :],
                                x_sb[:, nt, k, c0:c1],
                                start=(k == 0),
                                stop=(k == KC - 1),
                            )
                        if pool_copy and last and m2 == 1:
                            nc.scalar.copy(
                                o_all[:, nt, m2, c0:c1], pt[:, :c1 - c0])
                        else:
                            nc.vector.tensor_copy(
                                o_all[:, nt, m2, c0:c1], pt[:, :c1 - c0])

            # ---- output DMAs: partition-major dest, per-tile on Act
            oq = {"scalar": nc.scalar, "sync": nc.sync,
                  "pool": nc.gpsimd}[out_q]
            tq = [{"pool": nc.gpsimd, "scalar": nc.scalar,
                   "sync": nc.sync}[q] for q in tail_qs]
            ti = 0
            for nt in range(NTILES):
                last = nt == NTILES - 1
                if nt >= NTILES - 2:
                    pieces = ([(0, NT)] if not last else
                              [(0, NT - last_cols), (NT - last_cols, NT)])
                    for (c0, c1) in pieces:
                        for m2 in range(2):
                            tq[ti % len(tq)].dma_start(
                                y2.ap()[nt, :, m2, c0:c1],
                                o_all[:, nt, m2, c0:c1])
                            ti += 1
                else:
                    oq.dma_start(y2.ap()[nt], o_all[:, nt])

    nc.finalize()
    return nc

_NC_CACHE = None


def kernel(x, W1, b1, W2, b2):
    global _NC_CACHE
    x = np.asarray(x)
    W1, b1 = np.asarray(W1), np.asarray(b1)
    W2, b2 = np.asarray(W2), np.asarray(b2)
    n, c, h, w = x.shape  # 4, 64, 512, 512

    # ---- host: collapse the two linear maps exactly
    Wc = (W2.astype(np.float64) @ W1.astype(np.float64)).astype(np.float32)

    # ---- host unfold: cols[b, c*16+kh*4+kw, ph*128+pw] = x[b,c,ph*4+kh,pw*4+kw]
    xb = x.astype(_BF16)
    cols = xb.reshape(n, c, 128, 4, 128, 4).transpose(0, 1, 3, 5, 2, 4)
    cols = np.ascontiguousarray(cols).reshape(n, 1024, 16384)

    if _NC_CACHE is None:
        _NC_CACHE = _build_nc()
    nc = _NC_CACHE

    # wct[m2, p, k, m] = Wc[m2*128+m, k*128+p]
    wct6 = np.ascontiguousarray(
        Wc.astype(_BF16).reshape(2, P, KC, P).transpose(0, 3, 2, 1))

    in_maps = []
    for core in range(8):
        b, half = core // 2, core % 2
        # [1024, 8192] -> [16 nt, 128 p, 8 k, 512] (partition-major)
        xc = cols[b, :, half * LSH:(half + 1) * LSH]
        xc = np.ascontiguousarray(
            xc.reshape(KC, P, NTILES, NT).transpose(2, 1, 0, 3))
        in_maps.append({"xin": xc, "wct": wct6})

    res = run_bass_kernel_spmd(nc, in_maps, core_ids=list(range(8)))

    # ---- gather + fold on host
    y2 = np.empty((n, COUT, 16384), dtype=np.float32)
    for core in range(8):
        b, half = core // 2, core % 2
        yc = np.asarray(res.results[core]["y2"])
        # [16 nt, 128 p, 2 m2, 512] -> [m2, p, nt, n]
        yc = yc.astype(np.float32).transpose(2, 1, 0, 3)
        y2[b, :, half * LSH:(half + 1) * LSH] = yc.reshape(COUT, LSH)

    # bias epilogue (b1/b2 are zeros in this problem; exact otherwise)
    v = W2.astype(np.float64) @ b1.astype(np.float64) + b2.astype(np.float64)
    if np.any(v):
        y2 += v.astype(np.float32)[None, :, None]

    out = y2.reshape(n, c, 2, 2, 128, 128).transpose(0, 1, 4, 2, 5, 3)
    return np.ascontiguousarray(out).reshape(n, c, 256, 256)

